# revision 4
# baseline (speedup 1.0000x reference)
"""Trainium2 Bass kernel for nn_Attention (non-local-attention block + sync BN).

Computation per batch element b (B=8, C_IN=256, C_OUT=128, N=4096):
    theta = theta_w @ x + theta_b          [128, 4096]
    phi   = phi_w @ x + phi_b              [128, 4096]
    g     = g_w @ x + g_b                  [128, 4096]
    f     = theta^T @ phi / N              [4096, 4096]   (never materialized in DRAM)
    y     = g @ f^T                        [128, 4096]
    w_y   = W_w @ y  (+ W_b, cancels in BN)[256, 4096]
    out   = BN(w_y) * gamma + beta + x     (BN stats over all (B, N) -> AllReduce)

Sharding: data-parallel over batch across 8 NeuronCores (one element per
core); 1x1-conv weights replicated; BN batch stats synced with a tiny
[128,4] fp32 AllReduce.  Compute dtype bf16 (fp32 PSUM accumulation).
"""

import contextlib

import numpy as np
import ml_dtypes

import concourse.bass as bass  # noqa: F401  (registers engines)
import concourse.tile as tile
from concourse import bacc, mybir
from concourse import bass_utils

N_CORES = 8
B, C_IN, C_OUT, N = 8, 256, 128, 4096
P = 128
NCH = N // 512    # 8 column chunks of 512
MCH = N // 128    # 32 m-chunks of 128
BN_EPS = 1e-5

F32 = mybir.dt.float32
BF16 = mybir.dt.bfloat16
AF = mybir.ActivationFunctionType
ALU = mybir.AluOpType
AX = mybir.AxisListType


def _build_module():
    nc = bacc.Bacc("TRN2", target_bir_lowering=False, debug=False,
                   enable_asserts=True, num_devices=N_CORES)

    x32 = nc.dram_tensor("x32", [C_IN, N], F32, kind="ExternalInput").ap()
    x16 = nc.dram_tensor("x16", [C_IN, N], BF16, kind="ExternalInput").ap()
    thw = nc.dram_tensor("thw", [C_IN, C_OUT], BF16, kind="ExternalInput").ap()
    phw = nc.dram_tensor("phw", [C_IN, C_OUT], BF16, kind="ExternalInput").ap()
    gw = nc.dram_tensor("gw", [C_IN, C_OUT], BF16, kind="ExternalInput").ap()
    Ww = nc.dram_tensor("Ww", [C_OUT, C_IN], BF16, kind="ExternalInput").ap()
    thb = nc.dram_tensor("thb", [P, 1], F32, kind="ExternalInput").ap()
    phb = nc.dram_tensor("phb", [P, 1], F32, kind="ExternalInput").ap()
    gbb = nc.dram_tensor("gbb", [P, C_OUT], F32, kind="ExternalInput").ap()
    gam = nc.dram_tensor("gam", [P, 2], F32, kind="ExternalInput").ap()
    bet = nc.dram_tensor("bet", [P, 2], F32, kind="ExternalInput").ap()
    out = nc.dram_tensor("out", [C_IN, N], F32, kind="ExternalOutput").ap()

    with contextlib.ExitStack() as ctx:
        tc = ctx.enter_context(tile.TileContext(nc))
        pp = ctx.enter_context(tc.tile_pool(name="persist", bufs=1))
        ftsb = ctx.enter_context(tc.tile_pool(name="ftsb", bufs=3))
        ysb = ctx.enter_context(tc.tile_pool(name="ysb", bufs=2))
        sqp = ctx.enter_context(tc.tile_pool(name="sqp", bufs=2))
        op = ctx.enter_context(tc.tile_pool(name="outp", bufs=3))
        ps_cv = ctx.enter_context(tc.tile_pool(name="pscv", bufs=2, space="PSUM"))
        ps_ft = ctx.enter_context(tc.tile_pool(name="psft", bufs=2, space="PSUM"))
        ps_y = ctx.enter_context(tc.tile_pool(name="psy", bufs=2, space="PSUM"))
        dram = ctx.enter_context(tc.tile_pool(name="dram", bufs=1, space="DRAM"))

        # ---- persistent SBUF tensors ----
        x16h = [pp.tile([P, N], BF16, tag=f"x16_{h}", name=f"x16_{h}") for h in range(2)]
        x32h = [pp.tile([P, N], F32, tag=f"x32_{h}", name=f"x32_{h}") for h in range(2)]
        th_t = pp.tile([P, N], BF16, tag="th")
        ph_t = pp.tile([P, N], BF16, tag="ph")
        gt_t = pp.tile([P, N], BF16, tag="gt")       # g^T in 32 [128m x 128c] blocks
        wy_t = [pp.tile([P, N], F32, tag=f"wy{h}", name=f"wy{h}") for h in range(2)]
        stat_s = pp.tile([P, 16], F32, tag="stat_s")  # per-chunk sums
        stat_q = pp.tile([P, 16], F32, tag="stat_q")  # per-chunk sum-of-squares

        thw_t = [pp.tile([P, P], BF16, tag=f"thw{k}", name=f"thw{k}") for k in range(2)]
        phw_t = [pp.tile([P, P], BF16, tag=f"phw{k}", name=f"phw{k}") for k in range(2)]
        gw_t = [pp.tile([P, P], BF16, tag=f"gw{k}", name=f"gw{k}") for k in range(2)]
        Ww_t = pp.tile([P, C_IN], BF16, tag="Ww")
        thb_t = pp.tile([P, 1], F32, tag="thb")
        phb_t = pp.tile([P, 1], F32, tag="phb")
        gbb_t = pp.tile([P, C_OUT], F32, tag="gbb")
        gam_t = pp.tile([P, 2], F32, tag="gam")
        bet_t = pp.tile([P, 2], F32, tag="bet")

        for k in range(2):
            nc.sync.dma_start(thw_t[k][:], thw[k * P:(k + 1) * P, :])
            nc.sync.dma_start(phw_t[k][:], phw[k * P:(k + 1) * P, :])
            nc.sync.dma_start(gw_t[k][:], gw[k * P:(k + 1) * P, :])
        nc.sync.dma_start(Ww_t[:], Ww[:, :])
        nc.sync.dma_start(thb_t[:], thb[:, :])
        nc.sync.dma_start(phb_t[:], phb[:, :])
        nc.sync.dma_start(gbb_t[:], gbb[:, :])
        nc.sync.dma_start(gam_t[:], gam[:, :])
        nc.sync.dma_start(bet_t[:], bet[:, :])
        for h in range(2):
            nc.sync.dma_start(x16h[h][:], x16[h * P:(h + 1) * P, :])
            nc.sync.dma_start(x32h[h][:], x32[h * P:(h + 1) * P, :])

        def cs(i, w):  # column slice helper
            return slice(i * w, (i + 1) * w)

        # ---- phi conv: ph_t[:, j*512:+512] = phi_w @ x + phi_b (bf16) ----
        for j in range(NCH):
            ps = ps_cv.tile([P, 512], F32, tag="cv")
            nc.tensor.matmul(ps[:], phw_t[0][:], x16h[0][:, cs(j, 512)],
                             start=True, stop=False)
            nc.tensor.matmul(ps[:], phw_t[1][:], x16h[1][:, cs(j, 512)],
                             start=False, stop=True)
            nc.scalar.activation(ph_t[:, cs(j, 512)], ps[:], AF.Identity,
                                 bias=phb_t[:, 0:1])

        # ---- gT conv: gt_t[:, m*128:+128] = (g_w @ x)^T block + g_b ----
        for m in range(MCH):
            ps = ps_cv.tile([P, P], F32, tag="cv", name="psgt")
            nc.tensor.matmul(ps[:], x16h[0][:, cs(m, P)], gw_t[0][:],
                             start=True, stop=False)
            nc.tensor.matmul(ps[:], x16h[1][:, cs(m, P)], gw_t[1][:],
                             start=False, stop=True)
            nc.vector.tensor_tensor(gt_t[:, cs(m, P)], ps[:], gbb_t[:], op=ALU.add)

        # ---- main loop over n-chunks ----
        for j in range(NCH):
            # theta conv for this chunk (theta_w pre-scaled by 1/N on host)
            ps = ps_cv.tile([P, 512], F32, tag="cv")
            nc.tensor.matmul(ps[:], thw_t[0][:], x16h[0][:, cs(j, 512)],
                             start=True, stop=False)
            nc.tensor.matmul(ps[:], thw_t[1][:], x16h[1][:, cs(j, 512)],
                             start=False, stop=True)
            nc.scalar.activation(th_t[:, cs(j, 512)], ps[:], AF.Identity,
                                 bias=thb_t[:, 0:1])

            y_ps = ps_y.tile([P, 512], F32, tag="y")
            for k in range(MCH // 2):  # pairs of m-chunks
                ft_ps = ps_ft.tile([P, 1024], F32, tag="ft")
                nc.tensor.matmul(ft_ps[:, 0:512], ph_t[:, cs(2 * k, P)],
                                 th_t[:, cs(j, 512)], start=True, stop=True)
                nc.tensor.matmul(ft_ps[:, 512:1024], ph_t[:, cs(2 * k + 1, P)],
                                 th_t[:, cs(j, 512)], start=True, stop=True)
                ft_sb = ftsb.tile([P, 1024], BF16, tag="ft_sb")
                if (j * (MCH // 2) + k) % 2 == 0:
                    nc.vector.tensor_copy(ft_sb[:], ft_ps[:])
                else:
                    nc.scalar.activation(ft_sb[:], ft_ps[:], AF.Copy)
                nc.tensor.matmul(y_ps[:], gt_t[:, cs(2 * k, P)], ft_sb[:, 0:512],
                                 start=(k == 0), stop=False)
                nc.tensor.matmul(y_ps[:], gt_t[:, cs(2 * k + 1, P)],
                                 ft_sb[:, 512:1024], start=False,
                                 stop=(k == MCH // 2 - 1))
            y_sb = ysb.tile([P, 512], BF16, tag="y_sb")
            nc.vector.tensor_copy(y_sb[:], y_ps[:])

            for h in range(2):
                w_ps = ps_cv.tile([P, 512], F32, tag="cv")
                nc.tensor.matmul(w_ps[:], Ww_t[:, cs(h, P)], y_sb[:],
                                 start=True, stop=True)
                nc.scalar.activation(wy_t[h][:, cs(j, 512)], w_ps[:], AF.Copy)
                col = h * NCH + j
                nc.vector.reduce_sum(stat_s[:, col:col + 1], w_ps[:], axis=AX.X)
                sq = sqp.tile([P, 512], F32, tag="sq")
                nc.scalar.activation(sq[:], w_ps[:], AF.Square,
                                     accum_out=stat_q[:, col:col + 1])

        # ---- BN stats: local reduce, AllReduce, affine params ----
        s4 = pp.tile([P, 4], F32, tag="s4")
        nc.vector.reduce_sum(s4[:, 0:1], stat_s[:, 0:NCH], axis=AX.X)
        nc.vector.reduce_sum(s4[:, 1:2], stat_s[:, NCH:2 * NCH], axis=AX.X)
        nc.vector.reduce_sum(s4[:, 2:3], stat_q[:, 0:NCH], axis=AX.X)
        nc.vector.reduce_sum(s4[:, 3:4], stat_q[:, NCH:2 * NCH], axis=AX.X)
        in_b = dram.tile([P, 4], F32)
        out_b = dram.tile([P, 4], F32)
        nc.sync.dma_start(in_b[:], s4[:])
        nc.gpsimd.collective_compute(
            "AllReduce", ALU.add,
            replica_groups=[list(range(N_CORES))],
            ins=[in_b.opt()], outs=[out_b.opt()],
        )
        g4 = pp.tile([P, 4], F32, tag="g4")
        nc.sync.dma_start(g4[:], out_b[:])

        inv_cnt = 1.0 / (B * N)
        mn = pp.tile([P, 2], F32, tag="mn")
        ms = pp.tile([P, 2], F32, tag="ms")
        var = pp.tile([P, 2], F32, tag="var")
        tmp = pp.tile([P, 2], F32, tag="tmp")
        sd = pp.tile([P, 2], F32, tag="sd")
        rstd = pp.tile([P, 2], F32, tag="rstd")
        scl = pp.tile([P, 2], F32, tag="scl")
        bia = pp.tile([P, 2], F32, tag="bia")
        nc.vector.tensor_scalar_mul(mn[:], g4[:, 0:2], inv_cnt)
        nc.vector.tensor_scalar_mul(ms[:], g4[:, 2:4], inv_cnt)
        nc.vector.tensor_mul(tmp[:], mn[:], mn[:])
        nc.vector.tensor_sub(var[:], ms[:], tmp[:])
        nc.vector.tensor_scalar_add(var[:], var[:], BN_EPS)
        nc.scalar.activation(sd[:], var[:], AF.Sqrt)
        nc.vector.reciprocal(rstd[:], sd[:])
        nc.vector.tensor_mul(scl[:], rstd[:], gam_t[:])
        nc.vector.tensor_mul(tmp[:], mn[:], scl[:])
        nc.vector.tensor_sub(bia[:], bet_t[:], tmp[:])

        # ---- normalize + residual + store ----
        for h in range(2):
            for j in range(NCH):
                o1 = op.tile([P, 512], F32, tag="o1")
                nc.scalar.activation(o1[:], wy_t[h][:, cs(j, 512)], AF.Identity,
                                     bias=bia[:, h:h + 1], scale=scl[:, h:h + 1])
                o2 = op.tile([P, 512], F32, tag="o2")
                nc.vector.tensor_add(o2[:], o1[:], x32h[h][:, cs(j, 512)])
                nc.sync.dma_start(out[h * P:(h + 1) * P, cs(j, 512)], o2[:])

    nc.compile()
    return nc


_CACHE = {}


def _get_module():
    if "nc" not in _CACHE:
        _CACHE["nc"] = _build_module()
    return _CACHE["nc"]


def _prep_in_maps(x, g_w, g_b, theta_w, theta_b, phi_w, phi_b, W_w, W_b,
                  bn_gamma, bn_beta):
    bf = ml_dtypes.bfloat16
    f32 = np.float32
    x = np.ascontiguousarray(x, dtype=f32)
    shared = {
        "thw": np.ascontiguousarray((theta_w.T / N).astype(bf)),
        "phw": np.ascontiguousarray(phi_w.T.astype(bf)),
        "gw": np.ascontiguousarray(g_w.T.astype(bf)),
        "Ww": np.ascontiguousarray(W_w.T.astype(bf)),
        "thb": np.ascontiguousarray((theta_b / N).reshape(P, 1).astype(f32)),
        "phb": np.ascontiguousarray(phi_b.reshape(P, 1).astype(f32)),
        "gbb": np.ascontiguousarray(
            np.broadcast_to(g_b[None, :].astype(f32), (P, C_OUT))),
        "gam": np.ascontiguousarray(bn_gamma.reshape(2, P).T.astype(f32)),
        "bet": np.ascontiguousarray(bn_beta.reshape(2, P).T.astype(f32)),
    }
    in_maps = []
    for i in range(N_CORES):
        m = dict(shared)
        m["x32"] = x[i]
        m["x16"] = np.ascontiguousarray(x[i].astype(bf))
        in_maps.append(m)
    return in_maps


def _run(inputs, trace=False, trace_cores=None):
    nc = _get_module()
    in_maps = _prep_in_maps(**inputs)
    res = bass_utils.run_bass_kernel_spmd(
        nc, in_maps, core_ids=list(range(N_CORES)),
        trace=trace, trace_cores=trace_cores,
    )
    out = np.stack([res.results[i]["out"] for i in range(N_CORES)], axis=0)
    return out.astype(np.float32), res


def kernel(**inputs) -> np.ndarray:
    out, _ = _run(inputs, trace=False)
    return out


# revision 5
# speedup vs baseline: 1.1505x; 1.1505x over previous
"""Trainium2 Bass kernel for nn_Attention (non-local-attention block + sync BN).

Computation per batch element b (B=8, C_IN=256, C_OUT=128, N=4096):
    theta = theta_w @ x + theta_b          [128, 4096]
    phi   = phi_w @ x + phi_b              [128, 4096]
    g     = g_w @ x + g_b                  [128, 4096]
    f     = theta^T @ phi / N              [4096, 4096]   (never materialized in DRAM)
    y     = g @ f^T                        [128, 4096]
    w_y   = W_w @ y  (+ W_b, cancels in BN)[256, 4096]
    out   = BN(w_y) * gamma + beta + x     (BN stats over all (B, N) -> AllReduce)

Sharding: data-parallel over batch across 8 NeuronCores (one element per
core); 1x1-conv weights replicated; BN batch stats synced with a tiny
[128,4] fp32 AllReduce.  Compute dtype bf16 (fp32 PSUM accumulation).

Main loop is software-pipelined: the y-matmul for fT pair i is emitted
LAG iterations after the fT matmuls of pair i, so the PSUM->SBUF copy of
fT (split between the Vector and Scalar engines) overlaps with later fT
matmuls and the PE stream stays dense (keeps the HAM clock gate at 2.4GHz).
"""

import contextlib

import numpy as np
import ml_dtypes

import concourse.bass as bass  # noqa: F401  (registers engines)
import concourse.tile as tile
from concourse import bacc, mybir
from concourse import bass_utils

N_CORES = 8
B, C_IN, C_OUT, N = 8, 256, 128, 4096
P = 128
NCH = N // 512    # 8 column chunks of 512
MCH = N // 128    # 32 m-chunks of 128
KPAIR = MCH // 2  # 16 fT pairs per n-chunk
LAG = 2           # y-matmul lag (iterations) behind fT matmuls
BN_EPS = 1e-5

F32 = mybir.dt.float32
BF16 = mybir.dt.bfloat16
AF = mybir.ActivationFunctionType
ALU = mybir.AluOpType
AX = mybir.AxisListType


def _build_module():
    nc = bacc.Bacc("TRN2", target_bir_lowering=False, debug=False,
                   enable_asserts=True, num_devices=N_CORES)

    x32 = nc.dram_tensor("x32", [C_IN, N], F32, kind="ExternalInput").ap()
    x16 = nc.dram_tensor("x16", [C_IN, N], BF16, kind="ExternalInput").ap()
    thw = nc.dram_tensor("thw", [C_IN, C_OUT], BF16, kind="ExternalInput").ap()
    phw = nc.dram_tensor("phw", [C_IN, C_OUT], BF16, kind="ExternalInput").ap()
    gw = nc.dram_tensor("gw", [C_IN, C_OUT], BF16, kind="ExternalInput").ap()
    Ww = nc.dram_tensor("Ww", [C_OUT, C_IN], BF16, kind="ExternalInput").ap()
    thb = nc.dram_tensor("thb", [P, 1], F32, kind="ExternalInput").ap()
    phb = nc.dram_tensor("phb", [P, 1], F32, kind="ExternalInput").ap()
    gbb = nc.dram_tensor("gbb", [P, C_OUT], F32, kind="ExternalInput").ap()
    gam = nc.dram_tensor("gam", [P, 2], F32, kind="ExternalInput").ap()
    bet = nc.dram_tensor("bet", [P, 2], F32, kind="ExternalInput").ap()
    out = nc.dram_tensor("out", [C_IN, N], F32, kind="ExternalOutput").ap()

    with contextlib.ExitStack() as ctx:
        tc = ctx.enter_context(tile.TileContext(nc))
        pp = ctx.enter_context(tc.tile_pool(name="persist", bufs=1))
        ftsb = ctx.enter_context(tc.tile_pool(name="ftsb", bufs=4))
        ysb = ctx.enter_context(tc.tile_pool(name="ysb", bufs=2))
        sqp = ctx.enter_context(tc.tile_pool(name="sqp", bufs=2))
        op = ctx.enter_context(tc.tile_pool(name="outp", bufs=3))
        ps_cv = ctx.enter_context(tc.tile_pool(name="pscv", bufs=2, space="PSUM"))
        ps_ft = ctx.enter_context(tc.tile_pool(name="psft", bufs=2, space="PSUM"))
        ps_y = ctx.enter_context(tc.tile_pool(name="psy", bufs=2, space="PSUM"))
        dram = ctx.enter_context(tc.tile_pool(name="dram", bufs=1, space="DRAM"))

        # ---- persistent SBUF tensors ----
        x16h = [pp.tile([P, N], BF16, tag=f"x16_{h}", name=f"x16_{h}")
                for h in range(2)]
        x32h = [pp.tile([P, N], F32, tag=f"x32_{h}", name=f"x32_{h}")
                for h in range(2)]
        th_t = pp.tile([P, N], BF16, tag="th")
        ph_t = pp.tile([P, N], BF16, tag="ph")
        gt_t = pp.tile([P, N], BF16, tag="gt")       # g^T in 32 [128m x 128c] blocks
        wy_t = [pp.tile([P, N], F32, tag=f"wy{h}", name=f"wy{h}") for h in range(2)]
        stat_s = pp.tile([P, 16], F32, tag="stat_s")  # per-chunk sums
        stat_q = pp.tile([P, 16], F32, tag="stat_q")  # per-chunk sum-of-squares

        thw_t = [pp.tile([P, P], BF16, tag=f"thw{k}", name=f"thw{k}")
                 for k in range(2)]
        phw_t = [pp.tile([P, P], BF16, tag=f"phw{k}", name=f"phw{k}")
                 for k in range(2)]
        gw_t = [pp.tile([P, P], BF16, tag=f"gw{k}", name=f"gw{k}")
                for k in range(2)]
        Ww_t = pp.tile([P, C_IN], BF16, tag="Ww")
        thb_t = pp.tile([P, 1], F32, tag="thb")
        phb_t = pp.tile([P, 1], F32, tag="phb")
        gbb_t = pp.tile([P, C_OUT], F32, tag="gbb")
        gam_t = pp.tile([P, 2], F32, tag="gam")
        bet_t = pp.tile([P, 2], F32, tag="bet")

        def cs(i, w):  # column slice helper
            return slice(i * w, (i + 1) * w)

        # x16 DMA'd per 512-column chunk so the convs start early
        for j in range(NCH):
            for h in range(2):
                nc.sync.dma_start(x16h[h][:, cs(j, 512)],
                                  x16[h * P:(h + 1) * P, cs(j, 512)])
        for k in range(2):
            nc.sync.dma_start(thw_t[k][:], thw[k * P:(k + 1) * P, :])
            nc.sync.dma_start(phw_t[k][:], phw[k * P:(k + 1) * P, :])
            nc.sync.dma_start(gw_t[k][:], gw[k * P:(k + 1) * P, :])
        nc.sync.dma_start(Ww_t[:], Ww[:, :])
        nc.sync.dma_start(thb_t[:], thb[:, :])
        nc.sync.dma_start(phb_t[:], phb[:, :])
        nc.sync.dma_start(gbb_t[:], gbb[:, :])
        nc.sync.dma_start(gam_t[:], gam[:, :])
        nc.sync.dma_start(bet_t[:], bet[:, :])
        for h in range(2):
            nc.sync.dma_start(x32h[h][:], x32[h * P:(h + 1) * P, :])

        # ---- phi / theta convs (all chunks upfront) ----
        for (w_t, b_t, dst) in ((phw_t, phb_t, ph_t), (thw_t, thb_t, th_t)):
            for j in range(NCH):
                ps = ps_cv.tile([P, 512], F32, tag="cv", name="ps_conv")
                nc.tensor.matmul(ps[:], w_t[0][:], x16h[0][:, cs(j, 512)],
                                 start=True, stop=False)
                nc.tensor.matmul(ps[:], w_t[1][:], x16h[1][:, cs(j, 512)],
                                 start=False, stop=True)
                nc.scalar.activation(dst[:, cs(j, 512)], ps[:], AF.Identity,
                                     bias=b_t[:, 0:1])

        def emit_gt_conv(m):
            ps = ps_cv.tile([P, P], F32, tag="cv", name="ps_gt")
            nc.tensor.matmul(ps[:], x16h[0][:, cs(m, P)], gw_t[0][:],
                             start=True, stop=False)
            nc.tensor.matmul(ps[:], x16h[1][:, cs(m, P)], gw_t[1][:],
                             start=False, stop=True)
            nc.vector.tensor_tensor(gt_t[:, cs(m, P)], ps[:], gbb_t[:],
                                    op=ALU.add)

        def emit_w_block(j, y_sb):
            for h in range(2):
                w_ps = ps_cv.tile([P, 512], F32, tag="cv", name="ps_w")
                nc.tensor.matmul(w_ps[:], Ww_t[:, cs(h, P)], y_sb[:],
                                 start=True, stop=True)
                nc.scalar.activation(wy_t[h][:, cs(j, 512)], w_ps[:], AF.Copy)
                col = h * NCH + j
                nc.vector.reduce_sum(stat_s[:, col:col + 1],
                                     wy_t[h][:, cs(j, 512)], axis=AX.X)
                sq = sqp.tile([P, 512], F32, tag="sq", name="sq")
                nc.scalar.activation(sq[:], wy_t[h][:, cs(j, 512)], AF.Square,
                                     accum_out=stat_q[:, col:col + 1])

        # ---- software-pipelined main loop over flattened (j, k) pairs ----
        TOT = NCH * KPAIR  # 128
        ft_sbs = {}
        y_ps_cur = [None]
        pending_w = []  # (emit_at_iter, j, y_sb)

        for it in range(TOT + LAG):
            # gT convs embedded into the first iterations (2 per iter)
            if it < MCH // 2:
                emit_gt_conv(2 * it)
                emit_gt_conv(2 * it + 1)

            if it < TOT:
                j, k = divmod(it, KPAIR)
                ft_ps = ps_ft.tile([P, 1024], F32, tag="ft", name="ft_ps")
                nc.tensor.matmul(ft_ps[:, 0:512], ph_t[:, cs(2 * k, P)],
                                 th_t[:, cs(j, 512)], start=True, stop=True)
                nc.tensor.matmul(ft_ps[:, 512:1024], ph_t[:, cs(2 * k + 1, P)],
                                 th_t[:, cs(j, 512)], start=True, stop=True)
                ft_sb = ftsb.tile([P, 1024], BF16, tag="ft_sb", name="ft_sb")
                nc.vector.tensor_copy(ft_sb[:, 0:512], ft_ps[:, 0:512])
                nc.scalar.activation(ft_sb[:, 512:1024], ft_ps[:, 512:1024],
                                     AF.Copy)
                ft_sbs[it] = ft_sb

            while pending_w and pending_w[0][0] <= it:
                _, jw, y_sb_w = pending_w.pop(0)
                emit_w_block(jw, y_sb_w)

            iy = it - LAG
            if 0 <= iy < TOT:
                j2, k2 = divmod(iy, KPAIR)
                if k2 == 0:
                    y_ps_cur[0] = ps_y.tile([P, 512], F32, tag="y", name="y_ps")
                y_ps = y_ps_cur[0]
                ft_sb = ft_sbs.pop(iy)
                nc.tensor.matmul(y_ps[:], gt_t[:, cs(2 * k2, P)],
                                 ft_sb[:, 0:512], start=(k2 == 0), stop=False)
                nc.tensor.matmul(y_ps[:], gt_t[:, cs(2 * k2 + 1, P)],
                                 ft_sb[:, 512:1024], start=False,
                                 stop=(k2 == KPAIR - 1))
                if k2 == KPAIR - 1:
                    y_sb = ysb.tile([P, 512], BF16, tag="y_sb", name="y_sb")
                    nc.vector.tensor_copy(y_sb[:], y_ps[:])
                    pending_w.append((it + 2, j2, y_sb))

        while pending_w:
            _, jw, y_sb_w = pending_w.pop(0)
            emit_w_block(jw, y_sb_w)

        # ---- BN stats: local reduce, AllReduce, affine params ----
        s4 = pp.tile([P, 4], F32, tag="s4")
        nc.vector.reduce_sum(s4[:, 0:1], stat_s[:, 0:NCH], axis=AX.X)
        nc.vector.reduce_sum(s4[:, 1:2], stat_s[:, NCH:2 * NCH], axis=AX.X)
        nc.vector.reduce_sum(s4[:, 2:3], stat_q[:, 0:NCH], axis=AX.X)
        nc.vector.reduce_sum(s4[:, 3:4], stat_q[:, NCH:2 * NCH], axis=AX.X)
        in_b = dram.tile([P, 4], F32)
        out_b = dram.tile([P, 4], F32)
        nc.sync.dma_start(in_b[:], s4[:])
        nc.gpsimd.collective_compute(
            "AllReduce", ALU.add,
            replica_groups=[list(range(N_CORES))],
            ins=[in_b.opt()], outs=[out_b.opt()],
        )
        g4 = pp.tile([P, 4], F32, tag="g4")
        nc.sync.dma_start(g4[:], out_b[:])

        inv_cnt = 1.0 / (B * N)
        mn = pp.tile([P, 2], F32, tag="mn")
        ms = pp.tile([P, 2], F32, tag="ms")
        var = pp.tile([P, 2], F32, tag="var")
        tmp = pp.tile([P, 2], F32, tag="tmp")
        sd = pp.tile([P, 2], F32, tag="sd")
        rstd = pp.tile([P, 2], F32, tag="rstd")
        scl = pp.tile([P, 2], F32, tag="scl")
        bia = pp.tile([P, 2], F32, tag="bia")
        nc.vector.tensor_scalar_mul(mn[:], g4[:, 0:2], inv_cnt)
        nc.vector.tensor_scalar_mul(ms[:], g4[:, 2:4], inv_cnt)
        nc.vector.tensor_mul(tmp[:], mn[:], mn[:])
        nc.vector.tensor_sub(var[:], ms[:], tmp[:])
        nc.vector.tensor_scalar_add(var[:], var[:], BN_EPS)
        nc.scalar.activation(sd[:], var[:], AF.Sqrt)
        nc.vector.reciprocal(rstd[:], sd[:])
        nc.vector.tensor_mul(scl[:], rstd[:], gam_t[:])
        nc.vector.tensor_mul(tmp[:], mn[:], scl[:])
        nc.vector.tensor_sub(bia[:], bet_t[:], tmp[:])

        # ---- normalize + residual + store ----
        for h in range(2):
            for j in range(NCH):
                o1 = op.tile([P, 512], F32, tag="o1", name="o1")
                nc.scalar.activation(o1[:], wy_t[h][:, cs(j, 512)], AF.Identity,
                                     bias=bia[:, h:h + 1], scale=scl[:, h:h + 1])
                o2 = op.tile([P, 512], F32, tag="o2", name="o2")
                nc.vector.tensor_add(o2[:], o1[:], x32h[h][:, cs(j, 512)])
                nc.sync.dma_start(out[h * P:(h + 1) * P, cs(j, 512)], o2[:])

    nc.compile()
    return nc


_CACHE = {}


def _get_module():
    if "nc" not in _CACHE:
        _CACHE["nc"] = _build_module()
    return _CACHE["nc"]


def _prep_in_maps(x, g_w, g_b, theta_w, theta_b, phi_w, phi_b, W_w, W_b,
                  bn_gamma, bn_beta):
    bf = ml_dtypes.bfloat16
    f32 = np.float32
    x = np.ascontiguousarray(x, dtype=f32)
    shared = {
        "thw": np.ascontiguousarray((theta_w.T / N).astype(bf)),
        "phw": np.ascontiguousarray(phi_w.T.astype(bf)),
        "gw": np.ascontiguousarray(g_w.T.astype(bf)),
        "Ww": np.ascontiguousarray(W_w.T.astype(bf)),
        "thb": np.ascontiguousarray((theta_b / N).reshape(P, 1).astype(f32)),
        "phb": np.ascontiguousarray(phi_b.reshape(P, 1).astype(f32)),
        "gbb": np.ascontiguousarray(
            np.broadcast_to(g_b[None, :].astype(f32), (P, C_OUT))),
        "gam": np.ascontiguousarray(bn_gamma.reshape(2, P).T.astype(f32)),
        "bet": np.ascontiguousarray(bn_beta.reshape(2, P).T.astype(f32)),
    }
    in_maps = []
    for i in range(N_CORES):
        m = dict(shared)
        m["x32"] = x[i]
        m["x16"] = np.ascontiguousarray(x[i].astype(bf))
        in_maps.append(m)
    return in_maps


def _run(inputs, trace=False, trace_cores=None):
    nc = _get_module()
    in_maps = _prep_in_maps(**inputs)
    res = bass_utils.run_bass_kernel_spmd(
        nc, in_maps, core_ids=list(range(N_CORES)),
        trace=trace, trace_cores=trace_cores,
    )
    out = np.stack([res.results[i]["out"] for i in range(N_CORES)], axis=0)
    return out.astype(np.float32), res


def kernel(**inputs) -> np.ndarray:
    out, _ = _run(inputs, trace=False)
    return out


# revision 7
# speedup vs baseline: 1.3501x; 1.1736x over previous
"""Trainium2 Bass kernel for nn_Attention (non-local-attention block + sync BN).

Computation per batch element b (B=8, C_IN=256, C_OUT=128, N=4096):
    theta = theta_w @ x + theta_b          [128, 4096]
    phi   = phi_w @ x + phi_b              [128, 4096]
    g     = g_w @ x + g_b                  [128, 4096]
    f     = theta^T @ phi / N              [4096, 4096]   (never materialized in DRAM)
    y     = g @ f^T                        [128, 4096]
    w_y   = W_w @ y  (+ W_b, cancels in BN)[256, 4096]
    out   = BN(w_y) * gamma + beta + x     (BN stats over all (B, N) -> AllReduce)

Sharding: data-parallel over batch across 8 NeuronCores (one element per
core); 1x1-conv weights replicated; BN batch stats synced with a tiny
[128,4] fp32 AllReduce.  Compute dtype bf16 (fp32 PSUM accumulation).

Main loop is software-pipelined: the y-matmul for fT pair i is emitted
LAG iterations after the fT matmuls of pair i, so the PSUM->SBUF copy of
fT (split between the Vector and Scalar engines) overlaps with later fT
matmuls and the PE stream stays dense (keeps the HAM clock gate at 2.4GHz).
"""

import contextlib

import numpy as np
import ml_dtypes

import concourse.bass as bass  # noqa: F401  (registers engines)
import concourse.tile as tile
from concourse import bacc, mybir
from concourse import bass_utils

N_CORES = 8
B, C_IN, C_OUT, N = 8, 256, 128, 4096
P = 128
NCH = N // 512    # 8 column chunks of 512
MCH = N // 128    # 32 m-chunks of 128
KPAIR = MCH // 2  # 16 fT pairs per n-chunk
LAG = 2           # y-matmul lag (iterations) behind fT matmuls
BN_EPS = 1e-5

F32 = mybir.dt.float32
BF16 = mybir.dt.bfloat16
AF = mybir.ActivationFunctionType
ALU = mybir.AluOpType
AX = mybir.AxisListType


def _build_module():
    nc = bacc.Bacc("TRN2", target_bir_lowering=False, debug=False,
                   enable_asserts=True, num_devices=N_CORES)

    x32 = nc.dram_tensor("x32", [C_IN, N], F32, kind="ExternalInput").ap()
    x16 = nc.dram_tensor("x16", [C_IN, N], BF16, kind="ExternalInput").ap()
    # wpack columns: thw0 thw1 phw0 phw1 gw0 gw1 WwA WwB (8 x [128,128] bf16)
    wpack = nc.dram_tensor("wpack", [P, 1024], BF16, kind="ExternalInput").ap()
    # bpack columns: thb(1) phb(1) gam(2) bet(2) gbb(128)
    bpack = nc.dram_tensor("bpack", [P, 134], F32, kind="ExternalInput").ap()
    out = nc.dram_tensor("out", [C_IN, N], F32, kind="ExternalOutput").ap()

    with contextlib.ExitStack() as ctx:
        tc = ctx.enter_context(tile.TileContext(nc))
        pp = ctx.enter_context(tc.tile_pool(name="persist", bufs=1))
        ftsb = ctx.enter_context(tc.tile_pool(name="ftsb", bufs=4))
        ysb = ctx.enter_context(tc.tile_pool(name="ysb", bufs=2))
        sqp = ctx.enter_context(tc.tile_pool(name="sqp", bufs=2))
        op = ctx.enter_context(tc.tile_pool(name="outp", bufs=3))
        ps_cv = ctx.enter_context(tc.tile_pool(name="pscv", bufs=2, space="PSUM"))
        ps_ft = ctx.enter_context(tc.tile_pool(name="psft", bufs=2, space="PSUM"))
        ps_y = ctx.enter_context(tc.tile_pool(name="psy", bufs=2, space="PSUM"))
        dram = ctx.enter_context(tc.tile_pool(name="dram", bufs=1, space="DRAM"))

        # ---- persistent SBUF tensors ----
        x16h = [pp.tile([P, N], BF16, tag=f"x16_{h}", name=f"x16_{h}")
                for h in range(2)]
        x32h = [pp.tile([P, N], F32, tag=f"x32_{h}", name=f"x32_{h}")
                for h in range(2)]
        th_t = pp.tile([P, N], BF16, tag="th")
        ph_t = pp.tile([P, N], BF16, tag="ph")
        gt_t = pp.tile([P, N], BF16, tag="gt")       # g^T in 32 [128m x 128c] blocks
        wy_t = [pp.tile([P, N], F32, tag=f"wy{h}", name=f"wy{h}") for h in range(2)]
        stat_s = pp.tile([P, 16], F32, tag="stat_s")  # per-chunk sums
        stat_q = pp.tile([P, 16], F32, tag="stat_q")  # per-chunk sum-of-squares

        wp_t = pp.tile([P, 1024], BF16, tag="wp")
        bp_t = pp.tile([P, 134], F32, tag="bp")
        eps_t = pp.tile([P, 1], F32, tag="eps")
        nc.gpsimd.memset(eps_t[:], BN_EPS)

        def cs(i, w):  # column slice helper
            return slice(i * w, (i + 1) * w)

        # weight DMAs first (small), then x16 chunks so the convs start early,
        # x32 last via SWDGE (only needed for the tail residual)
        nc.sync.dma_start(wp_t[:], wpack[:, :])
        nc.sync.dma_start(bp_t[:], bpack[:, :])
        for j in range(NCH):
            for h in range(2):
                nc.sync.dma_start(x16h[h][:, cs(j, 512)],
                                  x16[h * P:(h + 1) * P, cs(j, 512)])
        for h in range(2):
            nc.gpsimd.dma_start(x32h[h][:], x32[h * P:(h + 1) * P, :])

        thw_t = [wp_t[:, cs(k, P)] for k in range(2)]
        phw_t = [wp_t[:, cs(2 + k, P)] for k in range(2)]
        gw_t = [wp_t[:, cs(4 + k, P)] for k in range(2)]
        Ww_h = [wp_t[:, cs(6 + h, P)] for h in range(2)]
        thb_t = bp_t[:, 0:1]
        phb_t = bp_t[:, 1:2]
        gam_t = bp_t[:, 2:4]
        bet_t = bp_t[:, 4:6]
        gbb_t = bp_t[:, 6:134]

        # ---- phi / theta convs, interleaved per chunk (DMA-paced) ----
        for j in range(NCH):
            for (w_t, b_t, dst) in ((phw_t, phb_t, ph_t), (thw_t, thb_t, th_t)):
                ps = ps_cv.tile([P, 512], F32, tag="cv", name="ps_conv")
                nc.tensor.matmul(ps[:], w_t[0], x16h[0][:, cs(j, 512)],
                                 start=True, stop=False)
                nc.tensor.matmul(ps[:], w_t[1], x16h[1][:, cs(j, 512)],
                                 start=False, stop=True)
                nc.scalar.activation(dst[:, cs(j, 512)], ps[:], AF.Identity,
                                     bias=b_t)

        def emit_gt_conv(m):
            ps = ps_cv.tile([P, P], F32, tag="cv", name="ps_gt")
            nc.tensor.matmul(ps[:], x16h[0][:, cs(m, P)], gw_t[0],
                             start=True, stop=False)
            nc.tensor.matmul(ps[:], x16h[1][:, cs(m, P)], gw_t[1],
                             start=False, stop=True)
            nc.vector.tensor_tensor(gt_t[:, cs(m, P)], ps[:], gbb_t[:],
                                    op=ALU.add)

        def emit_w_block(j, y_sb):
            for h in range(2):
                w_ps = ps_cv.tile([P, 512], F32, tag="cv", name="ps_w")
                nc.tensor.matmul(w_ps[:], Ww_h[h], y_sb[:],
                                 start=True, stop=True)
                nc.scalar.activation(wy_t[h][:, cs(j, 512)], w_ps[:], AF.Copy)
                col = h * NCH + j
                nc.vector.reduce_sum(stat_s[:, col:col + 1],
                                     wy_t[h][:, cs(j, 512)], axis=AX.X)
                sq = sqp.tile([P, 512], F32, tag="sq", name="sq")
                nc.scalar.activation(sq[:], wy_t[h][:, cs(j, 512)], AF.Square,
                                     accum_out=stat_q[:, col:col + 1])

        # ---- software-pipelined main loop over flattened (j, k) pairs ----
        TOT = NCH * KPAIR  # 128
        ft_sbs = {}
        y_ps_cur = [None]
        pending_w = []  # (emit_at_iter, j, y_sb)

        for it in range(TOT + LAG):
            # gT convs embedded into the first iterations (2 per iter)
            if it < MCH // 2:
                emit_gt_conv(2 * it)
                emit_gt_conv(2 * it + 1)

            if it < TOT:
                j, k = divmod(it, KPAIR)
                ft_ps = ps_ft.tile([P, 1024], F32, tag="ft", name="ft_ps")
                nc.tensor.matmul(ft_ps[:, 0:512], ph_t[:, cs(2 * k, P)],
                                 th_t[:, cs(j, 512)], start=True, stop=True)
                nc.tensor.matmul(ft_ps[:, 512:1024], ph_t[:, cs(2 * k + 1, P)],
                                 th_t[:, cs(j, 512)], start=True, stop=True)
                ft_sb = ftsb.tile([P, 1024], BF16, tag="ft_sb", name="ft_sb")
                if it % 2 == 0:
                    nc.vector.tensor_copy(ft_sb[:], ft_ps[:])
                else:
                    nc.scalar.activation(ft_sb[:], ft_ps[:], AF.Copy)
                ft_sbs[it] = ft_sb

            while pending_w and pending_w[0][0] <= it:
                _, jw, y_sb_w = pending_w.pop(0)
                emit_w_block(jw, y_sb_w)

            iy = it - LAG
            if 0 <= iy < TOT:
                j2, k2 = divmod(iy, KPAIR)
                if k2 == 0:
                    y_ps_cur[0] = ps_y.tile([P, 512], F32, tag="y", name="y_ps")
                y_ps = y_ps_cur[0]
                ft_sb = ft_sbs.pop(iy)
                nc.tensor.matmul(y_ps[:], gt_t[:, cs(2 * k2, P)],
                                 ft_sb[:, 0:512], start=(k2 == 0), stop=False)
                nc.tensor.matmul(y_ps[:], gt_t[:, cs(2 * k2 + 1, P)],
                                 ft_sb[:, 512:1024], start=False,
                                 stop=(k2 == KPAIR - 1))
                if k2 == KPAIR - 1:
                    y_sb = ysb.tile([P, 512], BF16, tag="y_sb", name="y_sb")
                    nc.vector.tensor_copy(y_sb[:], y_ps[:])
                    pending_w.append((it + 2, j2, y_sb))

        while pending_w:
            _, jw, y_sb_w = pending_w.pop(0)
            emit_w_block(jw, y_sb_w)

        # ---- BN stats: local reduce, AllReduce, affine params ----
        s4 = pp.tile([P, 4], F32, tag="s4")
        nc.vector.reduce_sum(s4[:, 0:1], stat_s[:, 0:NCH], axis=AX.X)
        nc.vector.reduce_sum(s4[:, 1:2], stat_s[:, NCH:2 * NCH], axis=AX.X)
        nc.vector.reduce_sum(s4[:, 2:3], stat_q[:, 0:NCH], axis=AX.X)
        nc.vector.reduce_sum(s4[:, 3:4], stat_q[:, NCH:2 * NCH], axis=AX.X)
        del stat_s, stat_q
        in_b = dram.tile([P, 4], F32)
        out_b = dram.tile([P, 4], F32)
        nc.sync.dma_start(in_b[:], s4[:])
        nc.gpsimd.collective_compute(
            "AllReduce", ALU.add,
            replica_groups=[list(range(N_CORES))],
            ins=[in_b.opt()], outs=[out_b.opt()],
        )
        g4 = pp.tile([P, 4], F32, tag="g4")
        nc.sync.dma_start(g4[:], out_b[:])

        inv_cnt = 1.0 / (B * N)
        m4 = pp.tile([P, 4], F32, tag="m4")      # [mn(2) | ms(2)]
        var = pp.tile([P, 2], F32, tag="var")
        tmp = pp.tile([P, 2], F32, tag="tmp")
        sd = pp.tile([P, 2], F32, tag="sd")
        rstd = pp.tile([P, 2], F32, tag="rstd")
        scl = pp.tile([P, 2], F32, tag="scl")
        bia = pp.tile([P, 2], F32, tag="bia")
        nc.vector.tensor_scalar_mul(m4[:], g4[:], inv_cnt)
        mn = m4[:, 0:2]
        nc.vector.tensor_mul(tmp[:], mn, mn)
        nc.vector.tensor_sub(var[:], m4[:, 2:4], tmp[:])
        nc.scalar.activation(sd[:], var[:], AF.Sqrt, bias=eps_t[:, 0:1])
        nc.vector.reciprocal(rstd[:], sd[:])
        nc.vector.tensor_mul(scl[:], rstd[:], gam_t)
        nc.vector.tensor_mul(tmp[:], mn, scl[:])
        nc.vector.tensor_sub(bia[:], bet_t, tmp[:])

        # ---- normalize + residual + store ----
        for idx in range(2 * NCH):
            h, j = divmod(idx, NCH)
            o1 = op.tile([P, 512], F32, tag="o1", name="o1")
            nc.scalar.activation(o1[:], wy_t[h][:, cs(j, 512)], AF.Identity,
                                 bias=bia[:, h:h + 1], scale=scl[:, h:h + 1])
            o2 = op.tile([P, 512], F32, tag="o2", name="o2")
            eng = nc.vector if idx % 2 == 0 else nc.gpsimd
            eng.tensor_add(o2[:], o1[:], x32h[h][:, cs(j, 512)])
            nc.sync.dma_start(out[h * P:(h + 1) * P, cs(j, 512)], o2[:])

    nc.compile()
    return nc


_CACHE = {}


def _get_module():
    if "nc" not in _CACHE:
        _CACHE["nc"] = _build_module()
    return _CACHE["nc"]


def _prep_in_maps(x, g_w, g_b, theta_w, theta_b, phi_w, phi_b, W_w, W_b,
                  bn_gamma, bn_beta):
    bf = ml_dtypes.bfloat16
    f32 = np.float32
    x = np.ascontiguousarray(x, dtype=f32)
    thwT = (theta_w.T / N).astype(bf)
    phwT = phi_w.T.astype(bf)
    gwT = g_w.T.astype(bf)
    WwT = W_w.T.astype(bf)
    wpack = np.concatenate(
        [thwT[0:P], thwT[P:2 * P], phwT[0:P], phwT[P:2 * P],
         gwT[0:P], gwT[P:2 * P], WwT[:, 0:P], WwT[:, P:2 * P]], axis=1)
    bpack = np.concatenate(
        [(theta_b / N).reshape(P, 1).astype(f32),
         phi_b.reshape(P, 1).astype(f32),
         bn_gamma.reshape(2, P).T.astype(f32),
         bn_beta.reshape(2, P).T.astype(f32),
         np.broadcast_to(g_b[None, :].astype(f32), (P, C_OUT))], axis=1)
    shared = {
        "wpack": np.ascontiguousarray(wpack),
        "bpack": np.ascontiguousarray(bpack),
    }
    in_maps = []
    for i in range(N_CORES):
        m = dict(shared)
        m["x32"] = x[i]
        m["x16"] = np.ascontiguousarray(x[i].astype(bf))
        in_maps.append(m)
    return in_maps


def _run(inputs, trace=False, trace_cores=None):
    nc = _get_module()
    in_maps = _prep_in_maps(**inputs)
    res = bass_utils.run_bass_kernel_spmd(
        nc, in_maps, core_ids=list(range(N_CORES)),
        trace=trace, trace_cores=trace_cores,
    )
    out = np.stack([res.results[i]["out"] for i in range(N_CORES)], axis=0)
    return out.astype(np.float32), res


def kernel(**inputs) -> np.ndarray:
    out, _ = _run(inputs, trace=False)
    return out
